# revision 22
# baseline (speedup 1.0000x reference)
"""Trainium2 Bass kernel for nn_DecoderLayer_66408784331382 (v4).

Single transformer decoder layer (RMSNorm + GQA attention w/ RoPE + RMSNorm +
SwiGLU MLP), tensor-parallel over 8 NeuronCores:

  - per core: 4 of 32 Q heads, 1 of 8 KV heads, 1024 of 8192 MLP inter cols,
    matching row-shards of wo / w_down.
  - activations transposed on device ([hid, tok]); host supplies x.T in fp16.
  - RMS1 folds *after* the QKV projection ((x*inv)@W == (x@W)*inv): QKV
    matmuls run on raw x; inv_rms folds into the RoPE cos/sin multipliers.
    The sum-of-squares stats for chunk c+1 are software-pipelined into chunk
    c's matmul stream so the inv_rms is ready the moment a chunk's PSUM
    accumulators complete — PSUM eviction never waits on the rms chain.
  - attention path is fp16, MLP bf16, PSUM accumulation fp32.  Softmax exp
    carries a -2.0 bias (cancels in normalization) for fp16 headroom.
  - V is laid out via XBAR DMA-transpose (no PE transposes / identity).
  - reciprocals use the approximate custom-DVE op on broadcast fp32 tiles.
  - the attention-output AllReduce is split into four 512-token fp16 chunks;
    a dummy 1-element AllReduce at kernel start absorbs the collective
    warm-up latency.  Emission is interleaved via generators: gate/up
    matmuls of MLP chunk c fill the PE while attention of later quarters
    waits on the exp chain; down-proj of chunk c overlaps gate/up of c+1.
  - o-proj accumulates across a 4-bank PSUM rotation with DVE evictions.
  - DMA queues: sync = streaming loads, scalar = weight preamble, vector =
    phase-1 rms broadcasts, gpsimd = gated traffic (collectives, softmax
    denominators, outputs).
  - down-proj partials (+ x1/8 residual) are written fp16, summed on host.

kernel(**inputs) takes the FULL fp32 inputs of reference.setup_inputs() and
returns the FULL [1, 2048, 2048] fp32 output.
"""

import sys

if "/opt/trn_rl_repo" not in sys.path:
    sys.path.insert(0, "/opt/trn_rl_repo")

import numpy as np
import ml_dtypes

import concourse.bass as bass
import concourse.mybir as mybir
import concourse.tile as tile
from concourse import bacc
from concourse.bass_utils import run_bass_kernel_spmd

# ---- problem constants (hardcoded per contract) ----
N_CORES = 8
S = 2048
HID = 2048
HD = 64
NH = 32
INTER = 8192
EPS = 1e-6

QD = (NH // N_CORES) * HD        # 256 local q cols (2 tiles of 128)
INTER_LOC = INTER // N_CORES     # 1024
SCALE = 1.0 / np.sqrt(HD)
EXPB = -2.0                      # softmax exp bias (cancels in normalization)

F32 = mybir.dt.float32
F32R = mybir.dt.float32r
BF16 = mybir.dt.bfloat16
F16 = mybir.dt.float16

P = 128
Q = 512      # token quarter (phase-1 chunk, attention block, MLP chunk)
NQ = S // Q  # 4
ARDT = F16   # collective dtype
AF = mybir.ActivationFunctionType
ALU = mybir.AluOpType


def _bcast(ap, parts):
    """View a [1, N] AP as [parts, N] via partition-stride-0 (DMA broadcast)."""
    return bass.AP(tensor=ap.tensor, offset=ap.offset,
                   ap=[[0, parts]] + [list(p) for p in ap.ap[1:]])


def _drive(*gens):
    """Round-robin the emission generators until all are exhausted."""
    active = [g for g in gens if g is not None]
    while active:
        for g in list(active):
            try:
                next(g)
            except StopIteration:
                active.remove(g)


def build():
    nc = bacc.Bacc("TRN2", target_bir_lowering=False, debug=False,
                   num_devices=N_CORES)

    hTb_d = nc.dram_tensor("hTb", [HID, S], F16, kind="ExternalInput")
    sin4_d = nc.dram_tensor("sin4", [P, S], F16, kind="ExternalInput")
    cos4_d = nc.dram_tensor("cos4", [P, S], F16, kind="ExternalInput")
    wq_d = nc.dram_tensor("wq", [HID, QD], F16, kind="ExternalInput")
    wkv_d = nc.dram_tensor("wkv", [HID, 2 * HD], F16, kind="ExternalInput")
    wo_d = nc.dram_tensor("wo", [QD, HID], F16, kind="ExternalInput")
    wg_d = nc.dram_tensor("wg", [HID, INTER_LOC], BF16, kind="ExternalInput")
    wu_d = nc.dram_tensor("wu", [HID, INTER_LOC], BF16, kind="ExternalInput")
    wd_d = nc.dram_tensor("wd", [INTER_LOC, HID], BF16, kind="ExternalInput")
    ones_d = nc.dram_tensor("ones", [P, 1], F32R, kind="ExternalInput")
    cvec_d = nc.dram_tensor("cvec", [P, 2], F32, kind="ExternalInput")
    masks_d = nc.dram_tensor("masks", [P, 4 * Q], F16, kind="ExternalInput")
    onesh_d = nc.dram_tensor("onesh", [P, 1], F16, kind="ExternalInput")
    outT_d = nc.dram_tensor("outT", [HID, S], F16, kind="ExternalOutput")

    with tile.TileContext(nc) as tc, nc.allow_low_precision(
            reason="f16/bf16 activations within a 2e-2 rel-err budget"):
        with (
            tc.tile_pool(name="const", bufs=1) as const,
            tc.tile_pool(name="dramp", bufs=1, space="DRAM") as dram,
        ):
            ones1 = const.tile([P, 1], F32R)
            eps1 = const.tile([P, 1], F32)
            expb1 = const.tile([P, 1], F32)
            nc.scalar.dma_start(ones1, ones_d[:, :])
            nc.scalar.dma_start(eps1, cvec_d[:, 0:1])
            nc.scalar.dma_start(expb1, cvec_d[:, 1:2])

            ar_in = [dram.tile([HID, Q], ARDT, name=f"ar_in{i}",
                               tag=f"ar_in{i}") for i in range(NQ)]
            ar_out = [dram.tile([HID, Q], ARDT, addr_space="Shared",
                                name=f"ar_out{i}", tag=f"ar_out{i}")
                      for i in range(NQ)]
            warm_in = dram.tile([1, 8], ARDT, tag="warm_in")
            warm_out = dram.tile([1, 8], ARDT, addr_space="Shared",
                                 tag="warm_out")
            bc1_dram = dram.tile([NQ, Q], F32)           # phase-1 rms rows
            bc2_dram = dram.tile([NQ, 2, 2, Q], F32)     # softmax denoms
            bc4_dram = dram.tile([NQ, Q], F32)           # rms2 rows

            # warm up the collective pipeline while phase 1 computes
            nc.gpsimd.collective_compute(
                "AllReduce", ALU.add,
                replica_groups=[list(range(N_CORES))],
                ins=[warm_in[:, :].opt()],
                outs=[warm_out[:, :].opt()])

            # ======== persistent tensors ===================================
            with tc.tile_pool(name="keep", bufs=1) as keep:
                masks = keep.tile([P, 4, Q], F16)
                nc.scalar.dma_start(
                    masks, masks_d[:, :].rearrange("p (t n) -> p t n", t=4))
                qT = [keep.tile([P, S], F16, tag=f"qT{m}", name=f"qT{m}")
                      for m in range(2)]
                kTdup = keep.tile([P, S], F16, tag="kTdup")
                v_ones = keep.tile([P, 16, HD + 1], F16, tag="v_ones")
                nc.scalar.dma_start(
                    v_ones[:, :, HD:HD + 1],
                    _bcast(bass.AP(tensor=onesh_d.tensor
                                   if hasattr(onesh_d, "tensor") else onesh_d,
                                   offset=0, ap=[[0, 1], [0, 16], [0, 1]]),
                           P))
                wo_all = keep.tile([P, 2, HID], F16, tag="wo_all")
                wd_all = keep.tile([P, 8, HID], BF16, tag="wd_all")

                # ---- Phase 1: QKV + pipelined RMS1 stats + RoPE -----------
                with (
                    tc.tile_pool(name="p1w", bufs=1) as p1w,
                    tc.tile_pool(name="p1x", bufs=2) as p1x,
                    tc.tile_pool(name="p1s", bufs=1) as p1s,
                    tc.tile_pool(name="p1ps", bufs=2, space="PSUM") as p1ps,
                ):
                    wq_all = p1w.tile([P, 16, QD], F16)
                    wkv_all = p1w.tile([P, 16, 2 * HD], F16)
                    nc.scalar.dma_start(
                        wq_all, wq_d[:, :].rearrange("(t p) m -> p t m", p=P))
                    nc.scalar.dma_start(
                        wkv_all, wkv_d[:, :].rearrange("(t p) m -> p t m", p=P))
                    sin4 = p1w.tile([P, S], F16)
                    cos4 = p1w.tile([P, S], F16)
                    nc.scalar.dma_start(sin4, sin4_d[:, :])
                    nc.scalar.dma_start(cos4, cos4_d[:, :])
                    # wo/wd after the phase-1 weights on the scalar queue
                    nc.scalar.dma_start(
                        wo_all, wo_d[:, :].rearrange("(t p) m -> p t m", p=P))
                    nc.scalar.dma_start(
                        wd_all, wd_d[:, :].rearrange("(t p) m -> p t m", p=P))

                    xqs, effs = {}, {}

                    def load_xq(c):
                        t = p1x.tile([P, 16, Q], F16, tag="xq")
                        cc = slice(Q * c, Q * (c + 1))
                        for t4 in range(4):
                            nc.sync.dma_start(
                                t[:, 4 * t4:4 * (t4 + 1), :],
                                hTb_d[512 * t4:512 * (t4 + 1), cc].rearrange(
                                    "(t p) m -> p t m", p=P))
                        xqs[c] = t

                    def stats_sq(c, kt, ssq):
                        """Square + ones-matmul accumulate for (c, kt)."""
                        sq = p1s.tile([P, Q], F32R, tag="sq", bufs=3)
                        nc.scalar.activation(sq, xqs[c][:, kt, :], AF.Square)
                        nc.tensor.matmul(ssq, ones1, sq,
                                         start=(kt == 0), stop=(kt == 15))

                    def rms_chain(c, ssq):
                        """Sqrt -> broadcast round-trip (vector queue) ->
                        approx reciprocal -> effective cos/sin."""
                        rms = p1s.tile([1, Q], F32, tag="rms", bufs=2)
                        nc.scalar.activation(rms, ssq, AF.Sqrt,
                                             bias=eps1[0:1, :], scale=1.0 / HID)
                        nc.sync.dma_start(bc1_dram[c:c + 1, :], rms)
                        rmsb = p1s.tile([P, Q], F32, tag="rmsb", bufs=2)
                        nc.sync.dma_start(rmsb,
                                          _bcast(bc1_dram[c:c + 1, :], P))
                        invb = p1s.tile([P, Q], F32, tag="invb", bufs=2)
                        nc.vector.reciprocal_approx_fast(invb, rmsb)
                        cc = slice(Q * c, Q * (c + 1))
                        cos_e = p1s.tile([P, Q], F16, tag="cos_e", bufs=2)
                        sin_e = p1s.tile([P, Q], F16, tag="sin_e", bufs=2)
                        nc.vector.tensor_mul(cos_e, cos4[:, cc], invb)
                        nc.vector.tensor_mul(sin_e, sin4[:, cc], invb)
                        effs[c] = (cos_e, sin_e, invb)

                    # pipeline fill: chunk-0 stats alone
                    load_xq(0)
                    load_xq(1)
                    ssq_c = p1ps.tile([1, Q], F32, tag="ssq")
                    for kt in range(16):
                        stats_sq(0, kt, ssq_c)
                    rms_chain(0, ssq_c)

                    for c in range(NQ):
                        cc = slice(Q * c, Q * (c + 1))
                        qm = p1ps.tile([P, 2 * Q], F32, tag="qm")
                        kv_ps = p1ps.tile([P, Q], F32, tag="kvps")
                        if c + 1 < NQ:
                            ssq_n = p1ps.tile([1, Q], F32, tag="ssq")
                        for kt in range(16):
                            st, sp = (kt == 0), (kt == 15)
                            xt = xqs[c][:, kt, :]
                            for m in range(2):
                                nc.tensor.matmul(
                                    qm[:, Q * m:Q * (m + 1)],
                                    wq_all[:, kt, P * m:P * (m + 1)],
                                    xt, start=st, stop=sp)
                            nc.tensor.matmul(kv_ps, wkv_all[:, kt, :],
                                             xt, start=st, stop=sp)
                            if c + 1 < NQ:
                                stats_sq(c + 1, kt, ssq_n)
                        if c + 2 < NQ:
                            load_xq(c + 2)
                        if c + 1 < NQ:
                            rms_chain(c + 1, ssq_n)
                        del xqs[c]

                        # RoPE eviction (inv_rms is ready: stats pipelined)
                        cos_e, sin_e, invb = effs.pop(c)
                        for m in range(2):
                            qp = qm[:, Q * m:Q * (m + 1)]
                            s1 = p1s.tile([P, Q], F16, tag="s1", bufs=2)
                            s2 = p1s.tile([P, Q], F16, tag="s2", bufs=2)
                            nc.vector.tensor_mul(s1, qp, cos_e)
                            for b in range(2):
                                x0 = slice(64 * b, 64 * b + 32)
                                x1s = slice(64 * b + 32, 64 * b + 64)
                                nc.vector.tensor_mul(
                                    s2[x0, :], qp[x1s, :], sin_e[x1s, :])
                                nc.vector.tensor_mul(
                                    s2[x1s, :], qp[x0, :], sin_e[x0, :])
                            nc.vector.tensor_add(qT[m][:, cc], s1, s2)
                        s1 = p1s.tile([64, Q], F16, tag="s1k", bufs=2)
                        s2 = p1s.tile([64, Q], F16, tag="s2k", bufs=2)
                        nc.vector.tensor_mul(s1, kv_ps[0:64, :],
                                             cos_e[0:64, :])
                        nc.vector.tensor_mul(
                            s2[0:32, :], kv_ps[32:64, :], sin_e[32:64, :])
                        nc.vector.tensor_mul(
                            s2[32:64, :], kv_ps[0:32, :], sin_e[0:32, :])
                        nc.vector.tensor_add(kTdup[0:64, cc], s1, s2)
                        nc.vector.tensor_copy(kTdup[64:128, cc],
                                              kTdup[0:64, cc])
                        # v: scale by inv, then XBAR DMA-transpose
                        vt = p1s.tile([64, Q], F16, tag="vt", bufs=2)
                        nc.vector.tensor_mul(vt, kv_ps[64:128, :],
                                             invb[0:64, :])
                        for j in range(Q // P):
                            vst = p1s.tile([P, HD], F16, tag="vst", bufs=2)
                            nc.sync.dma_start_transpose(
                                vst, vt[:, P * j:P * (j + 1)])
                            nc.vector.tensor_copy(
                                v_ones[:, (Q // P) * c + j, 0:HD], vst)

                # ---- Phases 2-4, interleaved via emission generators ------
                with (
                    tc.tile_pool(name="ps", bufs=1, space="PSUM") as psp,
                    tc.tile_pool(name="att", bufs=2) as att,
                    tc.tile_pool(name="mlp", bufs=2) as mlp,
                    tc.tile_pool(name="sc1", bufs=1) as sc1,
                ):
                    ps = [psp.tile([P, Q], F32, tag=f"ps{i}", name=f"ps{i}")
                          for i in range(8)]

                    def g_attn(qc4):
                        """Scores/softmax/PV per m, then o-proj, then AR."""
                        qs = slice(Q * qc4, Q * (qc4 + 1))
                        atn = []
                        for m in range(2):
                            pv = [ps[2 + 2 * m][0:HD + 1, :],
                                  ps[3 + 2 * m][0:HD + 1, :]]
                            nkt = 4 * qc4 + 4
                            for kt in range(nkt):
                                st, sp = (kt == 0), (kt == nkt - 1)
                                for b in range(2):
                                    rows = slice(64 * b, 64 * (b + 1))
                                    sc = ps[b][:, :]
                                    nc.tensor.matmul(
                                        sc,
                                        kTdup[rows, P * kt:P * (kt + 1)],
                                        qT[m][rows, qs],
                                        start=True, stop=True)
                                    pr = att.tile([P, Q], F16, tag=f"pr{b}",
                                                  bufs=2)
                                    nc.scalar.activation(
                                        pr, sc, AF.Exp, bias=expb1,
                                        scale=float(SCALE))
                                    if kt >= 4 * qc4:
                                        nc.vector.tensor_mul(
                                            pr, pr,
                                            masks[:, kt - 4 * qc4, :])
                                    nc.tensor.matmul(
                                        pv[b], v_ones[:, kt, :], pr,
                                        start=st, stop=sp)
                                yield
                            at = att.tile([P, Q], F16, tag=f"atn{m}")
                            atn.append(at)
                            for b in range(2):
                                den = att.tile([1, Q], F32, tag=f"den{b}")
                                nc.scalar.copy(den, pv[b][HD:HD + 1, :])
                                slot = bc2_dram[qc4:qc4 + 1, m, b, :]
                                nc.gpsimd.dma_start(slot, den)
                                recb = att.tile([64, Q], F32, tag=f"recb{b}",
                                                bufs=1)
                                nc.gpsimd.dma_start(recb, _bcast(slot, 64))
                                rec = att.tile([64, Q], F32, tag=f"rec{b}",
                                               bufs=1)
                                nc.vector.reciprocal_approx_fast(rec, recb)
                                nc.vector.tensor_mul(
                                    at[64 * b:64 * (b + 1), :],
                                    pv[b][0:HD, :], rec)
                            yield
                        for hm in range(16):
                            ops = ps[hm % 4][:, :]
                            for kt2 in range(2):
                                nc.tensor.matmul(
                                    ops,
                                    wo_all[:, kt2, P * hm:P * (hm + 1)],
                                    atn[kt2],
                                    start=(kt2 == 0), stop=(kt2 == 1))
                            osb = att.tile([P, Q], ARDT, tag="osb", bufs=2)
                            nc.vector.tensor_copy(osb, ops)
                            nc.gpsimd.dma_start(
                                ar_in[qc4][P * hm:P * (hm + 1), :], osb)
                            if hm % 2 == 1:
                                yield
                        nc.gpsimd.collective_compute(
                            "AllReduce", ALU.add,
                            replica_groups=[list(range(N_CORES))],
                            ins=[ar_in[qc4][:, :].opt()],
                            outs=[ar_out[qc4][:, :].opt()])

                    def g_pre(c):
                        """x1 = x + attn (f16), rms2 stats -> invb -> xn2."""
                        cs = slice(Q * c, Q * (c + 1))
                        x1 = mlp.tile([P, 16, Q], F16, tag="x1")
                        xn2 = mlp.tile([P, 16, Q], BF16, tag="xn2")
                        ssq2 = ps[4][0:1, :]
                        for kt in range(16):
                            rs = slice(P * kt, P * (kt + 1))
                            th = sc1.tile([P, Q], F16, tag="th", bufs=2)
                            ta = sc1.tile([P, Q], ARDT, tag="ta", bufs=2)
                            nc.sync.dma_start(th, hTb_d[rs, cs])
                            nc.gpsimd.dma_start(ta, ar_out[c][rs, :])
                            nc.vector.tensor_add(x1[:, kt, :], th, ta)
                            sq = sc1.tile([P, Q], F32R, tag="sq2", bufs=2)
                            nc.scalar.activation(sq, x1[:, kt, :], AF.Square)
                            nc.tensor.matmul(ssq2, ones1, sq,
                                             start=(kt == 0), stop=(kt == 15))
                            if kt % 4 == 3:
                                yield
                        rms = sc1.tile([1, Q], F32, tag="rms2", bufs=2)
                        nc.scalar.activation(rms, ssq2, AF.Sqrt,
                                             bias=eps1[0:1, :], scale=1.0 / HID)
                        nc.gpsimd.dma_start(bc4_dram[c:c + 1, :], rms)
                        rmsb = sc1.tile([P, Q], F32, tag="rmsb2", bufs=1)
                        nc.gpsimd.dma_start(rmsb,
                                            _bcast(bc4_dram[c:c + 1, :], P))
                        invb = sc1.tile([P, Q], F32, tag="invb2", bufs=1)
                        nc.vector.reciprocal_approx_fast(invb, rmsb)
                        for kt in range(16):
                            nc.vector.tensor_mul(xn2[:, kt, :], x1[:, kt, :],
                                                 invb)
                            if kt % 8 == 7:
                                yield
                        g_pre.out[c] = (x1, xn2)
                    g_pre.out = {}

                    def g_gu(c):
                        """gate/up matmuls for chunk c (PE + weight DMA only;
                        no scalar work, so it interleaves with exp safely)."""
                        x1, xn2 = g_pre.out[c]
                        for iq in range(8):
                            wg_t = mlp.tile([P, 16, P], BF16, tag="wgt",
                                            bufs=2)
                            wu_t = mlp.tile([P, 16, P], BF16, tag="wut",
                                            bufs=2)
                            nc.sync.dma_start(
                                wg_t, wg_d[:, P * iq:P * (iq + 1)].rearrange(
                                    "(t p) m -> p t m", p=P))
                            nc.sync.dma_start(
                                wu_t, wu_d[:, P * iq:P * (iq + 1)].rearrange(
                                    "(t p) m -> p t m", p=P))
                            gps = ps[6][:, :]
                            ups = ps[7][:, :]
                            for kt in range(16):
                                st, sp = (kt == 0), (kt == 15)
                                nc.tensor.matmul(gps, wg_t[:, kt, :],
                                                 xn2[:, kt, :],
                                                 start=st, stop=sp)
                                nc.tensor.matmul(ups, wu_t[:, kt, :],
                                                 xn2[:, kt, :],
                                                 start=st, stop=sp)
                                if kt % 2 == 1:
                                    yield
                            # evict raw gate/up on DVE (table-neutral);
                            # silu happens batched in g_fin.
                            graw = mlp.tile([P, Q], BF16, tag=f"graw{iq}",
                                            bufs=1)
                            nc.vector.tensor_copy(graw, gps)
                            upr = mlp.tile([P, Q], BF16, tag=f"upr{iq}",
                                           bufs=1)
                            nc.vector.tensor_copy(upr, ups)
                            g_gu.out[(c, iq)] = (graw, upr)
                            yield
                    g_gu.out = {}

                    def g_fin(c):
                        """silu + hmlp, then down-proj + residual + out DMA."""
                        x1, _ = g_pre.out[c]
                        hmlp = mlp.tile([P, 8, Q], BF16, tag="hmlp", bufs=1)
                        for iq in range(8):
                            graw, upr = g_gu.out.pop((c, iq))
                            sg = sc1.tile([P, Q], F16, tag="sg", bufs=1)
                            nc.scalar.activation(sg, graw, AF.Silu)
                            nc.vector.tensor_mul(hmlp[:, iq, :], sg, upr)
                            if iq % 4 == 3:
                                yield
                        for hm in range(16):
                            dps = ps[2 + (hm % 2)][:, :]
                            for kt8 in range(8):
                                nc.tensor.matmul(
                                    dps,
                                    wd_all[:, kt8, P * hm:P * (hm + 1)],
                                    hmlp[:, kt8, :],
                                    start=(kt8 == 0), stop=(kt8 == 7))
                            dsb = sc1.tile([P, Q], F16, tag="dsb", bufs=2)
                            nc.vector.scalar_tensor_tensor(
                                dsb, x1[:, hm, :], 1.0 / N_CORES, dps,
                                op0=ALU.mult, op1=ALU.add)
                            nc.gpsimd.dma_start(
                                outT_d[P * hm:P * (hm + 1),
                                       Q * c:Q * (c + 1)], dsb)
                            yield

                    # ---- interleaved emission schedule ----
                    _drive(g_attn(0))
                    _drive(g_attn(1))
                    _drive(g_attn(2))
                    _drive(g_pre(0))
                    _drive(g_attn(3), g_gu(0))
                    _drive(g_pre(1))
                    _drive(g_fin(0), g_gu(1))
                    _drive(g_pre(2))
                    _drive(g_fin(1), g_gu(2))
                    _drive(g_pre(3))
                    _drive(g_fin(2), g_gu(3))
                    _drive(g_fin(3))

    nc.compile()
    return nc


_CACHE = {}


def _get_nc():
    if "nc" not in _CACHE:
        _CACHE["nc"] = build()
    return _CACHE["nc"]


def _prep_inputs(inputs):
    """Shard + preprocess full inputs into 8 per-core in_maps."""
    f = lambda k: np.asarray(inputs[k], dtype=np.float32)
    hidden = f("hidden_states")[0]                 # [S, HID]
    sin_t, cos_t = f("sin_table"), f("cos_table")  # [S, 32]
    ln1, ln2 = f("ln1_w"), f("ln2_w")
    wq = (f("wq") * ln1[:, None]).astype(np.float16)
    wk = (f("wk") * ln1[:, None]).astype(np.float16)
    wv = (f("wv") * ln1[:, None]).astype(np.float16)
    wo = f("wo").astype(np.float16)
    wg = (f("w_gate") * ln2[:, None]).astype(ml_dtypes.bfloat16)
    wu = (f("w_up") * ln2[:, None]).astype(ml_dtypes.bfloat16)
    wd = f("w_down").astype(ml_dtypes.bfloat16)

    hTb = np.ascontiguousarray(hidden.T).astype(np.float16)
    # rows per 64-block: [+sinT (x0 source); -sinT (x1 source)]
    sin4 = np.ascontiguousarray(
        np.tile(np.concatenate([sin_t.T, -sin_t.T], axis=0),
                (2, 1))).astype(np.float16)
    cos4 = np.ascontiguousarray(np.tile(cos_t.T, (4, 1))).astype(np.float16)
    ones = np.ones((P, 1), dtype=np.float32)
    cvec = np.concatenate(
        [np.full((P, 1), EPS, np.float32), np.full((P, 1), EXPB, np.float32)],
        axis=1)
    onesh = np.ones((P, 1), dtype=np.float16)
    rr = np.arange(P)[:, None]
    cols = np.arange(Q)[None, :]
    masks = np.concatenate(
        [(rr + 128 * t <= cols).astype(np.float32) for t in range(4)],
        axis=1).astype(np.float16)

    in_maps = []
    for c in range(N_CORES):
        qs = slice(QD * c, QD * (c + 1))
        ks = slice(HD * c, HD * (c + 1))
        isl = slice(INTER_LOC * c, INTER_LOC * (c + 1))
        in_maps.append({
            "hTb": hTb,
            "sin4": sin4,
            "cos4": cos4,
            "wq": np.ascontiguousarray(wq[:, qs]),
            "wkv": np.ascontiguousarray(
                np.concatenate([wk[:, ks], wv[:, ks]], axis=1)),
            "wo": np.ascontiguousarray(wo[qs, :]),
            "wg": np.ascontiguousarray(wg[:, isl]),
            "wu": np.ascontiguousarray(wu[:, isl]),
            "wd": np.ascontiguousarray(wd[isl, :]),
            "ones": ones,
            "cvec": cvec,
            "onesh": onesh,
            "masks": masks,
        })
    return in_maps


def kernel(**inputs):
    nc = _get_nc()
    in_maps = _prep_inputs(inputs)
    res = run_bass_kernel_spmd(nc, in_maps, core_ids=list(range(N_CORES)))
    acc = np.zeros((HID, S), dtype=np.float32)
    for c in range(N_CORES):
        acc += res.results[c]["outT"].astype(np.float32)
    return np.ascontiguousarray(acc.T)[None, :, :]


# revision 23
# speedup vs baseline: 1.0298x; 1.0298x over previous
"""Trainium2 Bass kernel for nn_DecoderLayer_66408784331382 (v4).

Single transformer decoder layer (RMSNorm + GQA attention w/ RoPE + RMSNorm +
SwiGLU MLP), tensor-parallel over 8 NeuronCores:

  - per core: 4 of 32 Q heads, 1 of 8 KV heads, 1024 of 8192 MLP inter cols,
    matching row-shards of wo / w_down.
  - activations transposed on device ([hid, tok]); host supplies x.T in fp16.
  - RMS1 folds *after* the QKV projection ((x*inv)@W == (x@W)*inv): QKV
    matmuls run on raw x; inv_rms folds into the RoPE cos/sin multipliers.
    The sum-of-squares stats for chunk c+1 are software-pipelined into chunk
    c's matmul stream so the inv_rms is ready the moment a chunk's PSUM
    accumulators complete — PSUM eviction never waits on the rms chain.
  - attention path is fp16, MLP bf16, PSUM accumulation fp32.  Softmax exp
    carries a -2.0 bias (cancels in normalization) for fp16 headroom.
  - V is laid out via XBAR DMA-transpose (no PE transposes / identity).
  - reciprocals use the approximate custom-DVE op on broadcast fp32 tiles.
  - the attention-output AllReduce is split into four 512-token fp16 chunks;
    a dummy 1-element AllReduce at kernel start absorbs the collective
    warm-up latency.  Emission is interleaved via generators: gate/up
    matmuls of MLP chunk c fill the PE while attention of later quarters
    waits on the exp chain; down-proj of chunk c overlaps gate/up of c+1.
  - o-proj accumulates across a 4-bank PSUM rotation with DVE evictions.
  - DMA queues: sync = streaming loads, scalar = weight preamble, vector =
    phase-1 rms broadcasts, gpsimd = gated traffic (collectives, softmax
    denominators, outputs).
  - down-proj partials (+ x1/8 residual) are written fp16, summed on host.

kernel(**inputs) takes the FULL fp32 inputs of reference.setup_inputs() and
returns the FULL [1, 2048, 2048] fp32 output.
"""

import sys

if "/opt/trn_rl_repo" not in sys.path:
    sys.path.insert(0, "/opt/trn_rl_repo")

import numpy as np
import ml_dtypes

import concourse.bass as bass
import concourse.mybir as mybir
import concourse.tile as tile
from concourse import bacc
from concourse.bass_utils import run_bass_kernel_spmd

# ---- problem constants (hardcoded per contract) ----
N_CORES = 8
S = 2048
HID = 2048
HD = 64
NH = 32
INTER = 8192
EPS = 1e-6

QD = (NH // N_CORES) * HD        # 256 local q cols (2 tiles of 128)
INTER_LOC = INTER // N_CORES     # 1024
SCALE = 1.0 / np.sqrt(HD)
EXPB = -2.0                      # softmax exp bias (cancels in normalization)

F32 = mybir.dt.float32
F32R = mybir.dt.float32r
BF16 = mybir.dt.bfloat16
F16 = mybir.dt.float16

P = 128
Q = 512      # token quarter (phase-1 chunk, attention block, MLP chunk)
NQ = S // Q  # 4
ARDT = F16   # collective dtype
AF = mybir.ActivationFunctionType
ALU = mybir.AluOpType


def _bcast(ap, parts):
    """View a [1, N] AP as [parts, N] via partition-stride-0 (DMA broadcast)."""
    return bass.AP(tensor=ap.tensor, offset=ap.offset,
                   ap=[[0, parts]] + [list(p) for p in ap.ap[1:]])


def _drive(*gens):
    """Round-robin the emission generators until all are exhausted."""
    active = [g for g in gens if g is not None]
    while active:
        for g in list(active):
            try:
                next(g)
            except StopIteration:
                active.remove(g)


def build():
    nc = bacc.Bacc("TRN2", target_bir_lowering=False, debug=False,
                   num_devices=N_CORES)

    hTb_d = nc.dram_tensor("hTb", [HID, S], F16, kind="ExternalInput")
    sin4_d = nc.dram_tensor("sin4", [P, S], F16, kind="ExternalInput")
    cos4_d = nc.dram_tensor("cos4", [P, S], F16, kind="ExternalInput")
    wq_d = nc.dram_tensor("wq", [HID, QD], F16, kind="ExternalInput")
    wkv_d = nc.dram_tensor("wkv", [HID, 2 * HD], F16, kind="ExternalInput")
    wo_d = nc.dram_tensor("wo", [QD, HID], F16, kind="ExternalInput")
    wg_d = nc.dram_tensor("wg", [HID, INTER_LOC], BF16, kind="ExternalInput")
    wu_d = nc.dram_tensor("wu", [HID, INTER_LOC], BF16, kind="ExternalInput")
    wd_d = nc.dram_tensor("wd", [INTER_LOC, HID], BF16, kind="ExternalInput")
    ones_d = nc.dram_tensor("ones", [P, 1], F32R, kind="ExternalInput")
    cvec_d = nc.dram_tensor("cvec", [P, 2], F32, kind="ExternalInput")
    masks_d = nc.dram_tensor("masks", [P, 4 * Q], F16, kind="ExternalInput")
    onesh_d = nc.dram_tensor("onesh", [P, 1], F16, kind="ExternalInput")
    outT_d = nc.dram_tensor("outT", [HID, S], F16, kind="ExternalOutput")

    with tile.TileContext(nc) as tc, nc.allow_low_precision(
            reason="f16/bf16 activations within a 2e-2 rel-err budget"):
        with (
            tc.tile_pool(name="const", bufs=1) as const,
            tc.tile_pool(name="dramp", bufs=1, space="DRAM") as dram,
        ):
            ones1 = const.tile([P, 1], F32R)
            eps1 = const.tile([P, 1], F32)
            expb1 = const.tile([P, 1], F32)
            nc.scalar.dma_start(ones1, ones_d[:, :])
            nc.scalar.dma_start(eps1, cvec_d[:, 0:1])
            nc.scalar.dma_start(expb1, cvec_d[:, 1:2])

            ar_in = [dram.tile([HID, Q], ARDT, name=f"ar_in{i}",
                               tag=f"ar_in{i}") for i in range(NQ)]
            ar_out = [dram.tile([HID, Q], ARDT, addr_space="Shared",
                                name=f"ar_out{i}", tag=f"ar_out{i}")
                      for i in range(NQ)]
            warm_in = dram.tile([1, 8], ARDT, tag="warm_in")
            warm_out = dram.tile([1, 8], ARDT, addr_space="Shared",
                                 tag="warm_out")
            bc1_dram = dram.tile([NQ, Q], F32)           # phase-1 rms rows
            bc2_dram = dram.tile([NQ, 2, 2, Q], F32)     # softmax denoms
            bc4_dram = dram.tile([NQ, Q], F32)           # rms2 rows

            # warm up the collective pipeline while phase 1 computes
            nc.gpsimd.collective_compute(
                "AllReduce", ALU.add,
                replica_groups=[list(range(N_CORES))],
                ins=[warm_in[:, :].opt()],
                outs=[warm_out[:, :].opt()])

            # ======== persistent tensors ===================================
            with tc.tile_pool(name="keep", bufs=1) as keep:
                masks = keep.tile([P, 4, Q], F16)
                nc.scalar.dma_start(
                    masks, masks_d[:, :].rearrange("p (t n) -> p t n", t=4))
                qT = [keep.tile([P, S], F16, tag=f"qT{m}", name=f"qT{m}")
                      for m in range(2)]
                kTdup = keep.tile([P, S], F16, tag="kTdup")
                v_ones = keep.tile([P, 16, HD + 1], F16, tag="v_ones")
                nc.scalar.dma_start(
                    v_ones[:, :, HD:HD + 1],
                    _bcast(bass.AP(tensor=onesh_d.tensor
                                   if hasattr(onesh_d, "tensor") else onesh_d,
                                   offset=0, ap=[[0, 1], [0, 16], [0, 1]]),
                           P))
                wo_all = keep.tile([P, 2, HID], F16, tag="wo_all")
                wd_all = keep.tile([P, 8, HID], BF16, tag="wd_all")

                # ---- Phase 1: QKV + pipelined RMS1 stats + RoPE -----------
                with (
                    tc.tile_pool(name="p1w", bufs=1) as p1w,
                    tc.tile_pool(name="p1x", bufs=2) as p1x,
                    tc.tile_pool(name="p1s", bufs=1) as p1s,
                    tc.tile_pool(name="p1ps", bufs=2, space="PSUM") as p1ps,
                ):
                    wq_all = p1w.tile([P, 16, QD], F16)
                    wkv_all = p1w.tile([P, 16, 2 * HD], F16)
                    nc.scalar.dma_start(
                        wq_all, wq_d[:, :].rearrange("(t p) m -> p t m", p=P))
                    nc.scalar.dma_start(
                        wkv_all, wkv_d[:, :].rearrange("(t p) m -> p t m", p=P))
                    sin4 = p1w.tile([P, S], F16)
                    cos4 = p1w.tile([P, S], F16)
                    nc.scalar.dma_start(sin4, sin4_d[:, :])
                    nc.scalar.dma_start(cos4, cos4_d[:, :])
                    xqs, effs = {}, {}

                    def load_xq(c):
                        t = p1x.tile([P, 16, Q], F16, tag="xq")
                        cc = slice(Q * c, Q * (c + 1))
                        for t4 in range(4):
                            nc.sync.dma_start(
                                t[:, 4 * t4:4 * (t4 + 1), :],
                                hTb_d[512 * t4:512 * (t4 + 1), cc].rearrange(
                                    "(t p) m -> p t m", p=P))
                        xqs[c] = t

                    def stats_sq(c, kt, ssq):
                        """Square + ones-matmul accumulate for (c, kt)."""
                        sq = p1s.tile([P, Q], F32R, tag="sq", bufs=3)
                        nc.scalar.activation(sq, xqs[c][:, kt, :], AF.Square)
                        nc.tensor.matmul(ssq, ones1, sq,
                                         start=(kt == 0), stop=(kt == 15))

                    def rms_chain(c, ssq):
                        """Sqrt -> broadcast round-trip (vector queue) ->
                        approx reciprocal -> effective cos/sin."""
                        rms = p1s.tile([1, Q], F32, tag="rms", bufs=2)
                        nc.scalar.activation(rms, ssq, AF.Sqrt,
                                             bias=eps1[0:1, :], scale=1.0 / HID)
                        nc.sync.dma_start(bc1_dram[c:c + 1, :], rms)
                        rmsb = p1s.tile([P, Q], F32, tag="rmsb", bufs=2)
                        nc.sync.dma_start(rmsb,
                                          _bcast(bc1_dram[c:c + 1, :], P))
                        invb = p1s.tile([P, Q], F32, tag="invb", bufs=2)
                        nc.vector.reciprocal_approx_fast(invb, rmsb)
                        cc = slice(Q * c, Q * (c + 1))
                        cos_e = p1s.tile([P, Q], F16, tag="cos_e", bufs=2)
                        sin_e = p1s.tile([P, Q], F16, tag="sin_e", bufs=2)
                        nc.vector.tensor_mul(cos_e, cos4[:, cc], invb)
                        nc.vector.tensor_mul(sin_e, sin4[:, cc], invb)
                        effs[c] = (cos_e, sin_e, invb)

                    # pipeline fill: chunk-0 stats alone
                    load_xq(0)
                    load_xq(1)
                    ssq_c = p1ps.tile([1, Q], F32, tag="ssq")
                    for kt in range(16):
                        stats_sq(0, kt, ssq_c)
                    rms_chain(0, ssq_c)

                    for c in range(NQ):
                        cc = slice(Q * c, Q * (c + 1))
                        qm = p1ps.tile([P, 2 * Q], F32, tag="qm")
                        kv_ps = p1ps.tile([P, Q], F32, tag="kvps")
                        if c + 1 < NQ:
                            ssq_n = p1ps.tile([1, Q], F32, tag="ssq")
                        for kt in range(16):
                            st, sp = (kt == 0), (kt == 15)
                            xt = xqs[c][:, kt, :]
                            for m in range(2):
                                nc.tensor.matmul(
                                    qm[:, Q * m:Q * (m + 1)],
                                    wq_all[:, kt, P * m:P * (m + 1)],
                                    xt, start=st, stop=sp)
                            nc.tensor.matmul(kv_ps, wkv_all[:, kt, :],
                                             xt, start=st, stop=sp)
                            if c + 1 < NQ:
                                stats_sq(c + 1, kt, ssq_n)
                        if c + 2 < NQ:
                            load_xq(c + 2)
                        if c + 1 < NQ:
                            rms_chain(c + 1, ssq_n)
                        del xqs[c]

                        # RoPE eviction (inv_rms is ready: stats pipelined)
                        cos_e, sin_e, invb = effs.pop(c)
                        for m in range(2):
                            qp = qm[:, Q * m:Q * (m + 1)]
                            s1 = p1s.tile([P, Q], F16, tag="s1", bufs=2)
                            s2 = p1s.tile([P, Q], F16, tag="s2", bufs=2)
                            nc.vector.tensor_mul(s1, qp, cos_e)
                            for b in range(2):
                                x0 = slice(64 * b, 64 * b + 32)
                                x1s = slice(64 * b + 32, 64 * b + 64)
                                nc.vector.tensor_mul(
                                    s2[x0, :], qp[x1s, :], sin_e[x1s, :])
                                nc.vector.tensor_mul(
                                    s2[x1s, :], qp[x0, :], sin_e[x0, :])
                            nc.vector.tensor_add(qT[m][:, cc], s1, s2)
                        s1 = p1s.tile([64, Q], F16, tag="s1k", bufs=2)
                        s2 = p1s.tile([64, Q], F16, tag="s2k", bufs=2)
                        nc.vector.tensor_mul(s1, kv_ps[0:64, :],
                                             cos_e[0:64, :])
                        nc.vector.tensor_mul(
                            s2[0:32, :], kv_ps[32:64, :], sin_e[32:64, :])
                        nc.vector.tensor_mul(
                            s2[32:64, :], kv_ps[0:32, :], sin_e[0:32, :])
                        nc.vector.tensor_add(kTdup[0:64, cc], s1, s2)
                        nc.vector.tensor_copy(kTdup[64:128, cc],
                                              kTdup[0:64, cc])
                        # v: scale by inv, then XBAR DMA-transpose
                        vt = p1s.tile([64, Q], F16, tag="vt", bufs=2)
                        nc.vector.tensor_mul(vt, kv_ps[64:128, :],
                                             invb[0:64, :])
                        for j in range(Q // P):
                            vst = p1s.tile([P, HD], F16, tag="vst", bufs=2)
                            nc.sync.dma_start_transpose(
                                vst, vt[:, P * j:P * (j + 1)])
                            nc.vector.tensor_copy(
                                v_ones[:, (Q // P) * c + j, 0:HD], vst)

                nc.scalar.dma_start(
                    wo_all, wo_d[:, :].rearrange("(t p) m -> p t m", p=P))
                nc.scalar.dma_start(
                    wd_all, wd_d[:, :].rearrange("(t p) m -> p t m", p=P))

                # ---- Phases 2-4, interleaved via emission generators ------
                with (
                    tc.tile_pool(name="ps", bufs=1, space="PSUM") as psp,
                    tc.tile_pool(name="att", bufs=2) as att,
                    tc.tile_pool(name="mlp", bufs=2) as mlp,
                    tc.tile_pool(name="sc1", bufs=1) as sc1,
                ):
                    ps = [psp.tile([P, Q], F32, tag=f"ps{i}", name=f"ps{i}")
                          for i in range(8)]

                    def g_attn(qc4):
                        """Scores/softmax/PV per m, then o-proj, then AR."""
                        qs = slice(Q * qc4, Q * (qc4 + 1))
                        atn = []
                        for m in range(2):
                            pv = [ps[2 + 2 * m][0:HD + 1, :],
                                  ps[3 + 2 * m][0:HD + 1, :]]
                            nkt = 4 * qc4 + 4
                            for kt in range(nkt):
                                st, sp = (kt == 0), (kt == nkt - 1)
                                for b in range(2):
                                    rows = slice(64 * b, 64 * (b + 1))
                                    sc = ps[b][:, :]
                                    nc.tensor.matmul(
                                        sc,
                                        kTdup[rows, P * kt:P * (kt + 1)],
                                        qT[m][rows, qs],
                                        start=True, stop=True)
                                    pr = att.tile([P, Q], F16, tag=f"pr{b}",
                                                  bufs=2)
                                    nc.scalar.activation(
                                        pr, sc, AF.Exp, bias=expb1,
                                        scale=float(SCALE))
                                    if kt >= 4 * qc4:
                                        nc.vector.tensor_mul(
                                            pr, pr,
                                            masks[:, kt - 4 * qc4, :])
                                    nc.tensor.matmul(
                                        pv[b], v_ones[:, kt, :], pr,
                                        start=st, stop=sp)
                                yield
                            at = att.tile([P, Q], F16, tag=f"atn{m}")
                            atn.append(at)
                            for b in range(2):
                                den = att.tile([1, Q], F32, tag=f"den{b}")
                                nc.scalar.copy(den, pv[b][HD:HD + 1, :])
                                slot = bc2_dram[qc4:qc4 + 1, m, b, :]
                                nc.sync.dma_start(slot, den)
                                recb = att.tile([64, Q], F32, tag=f"recb{b}",
                                                bufs=1)
                                nc.sync.dma_start(recb, _bcast(slot, 64))
                                rec = att.tile([64, Q], F32, tag=f"rec{b}",
                                               bufs=1)
                                nc.vector.reciprocal_approx_fast(rec, recb)
                                nc.vector.tensor_mul(
                                    at[64 * b:64 * (b + 1), :],
                                    pv[b][0:HD, :], rec)
                            yield
                        for hm in range(16):
                            ops = ps[hm % 4][:, :]
                            for kt2 in range(2):
                                nc.tensor.matmul(
                                    ops,
                                    wo_all[:, kt2, P * hm:P * (hm + 1)],
                                    atn[kt2],
                                    start=(kt2 == 0), stop=(kt2 == 1))
                            osb = att.tile([P, Q], ARDT, tag="osb", bufs=2)
                            nc.vector.tensor_copy(osb, ops)
                            nc.sync.dma_start(
                                ar_in[qc4][P * hm:P * (hm + 1), :], osb)
                            if hm % 2 == 1:
                                yield
                        nc.gpsimd.collective_compute(
                            "AllReduce", ALU.add,
                            replica_groups=[list(range(N_CORES))],
                            ins=[ar_in[qc4][:, :].opt()],
                            outs=[ar_out[qc4][:, :].opt()])

                    def g_pre(c):
                        """x1 = x + attn (f16), rms2 stats -> invb -> xn2."""
                        cs = slice(Q * c, Q * (c + 1))
                        x1 = mlp.tile([P, 16, Q], F16, tag="x1")
                        xn2 = mlp.tile([P, 16, Q], BF16, tag="xn2")
                        ssq2 = ps[4][0:1, :]
                        for kt in range(16):
                            rs = slice(P * kt, P * (kt + 1))
                            th = sc1.tile([P, Q], F16, tag="th", bufs=2)
                            ta = sc1.tile([P, Q], ARDT, tag="ta", bufs=2)
                            nc.sync.dma_start(th, hTb_d[rs, cs])
                            nc.gpsimd.dma_start(ta, ar_out[c][rs, :])
                            nc.vector.tensor_add(x1[:, kt, :], th, ta)
                            sq = sc1.tile([P, Q], F32R, tag="sq2", bufs=2)
                            nc.scalar.activation(sq, x1[:, kt, :], AF.Square)
                            nc.tensor.matmul(ssq2, ones1, sq,
                                             start=(kt == 0), stop=(kt == 15))
                            if kt % 4 == 3:
                                yield
                        rms = sc1.tile([1, Q], F32, tag="rms2", bufs=2)
                        nc.scalar.activation(rms, ssq2, AF.Sqrt,
                                             bias=eps1[0:1, :], scale=1.0 / HID)
                        nc.gpsimd.dma_start(bc4_dram[c:c + 1, :], rms)
                        rmsb = sc1.tile([P, Q], F32, tag="rmsb2", bufs=1)
                        nc.gpsimd.dma_start(rmsb,
                                            _bcast(bc4_dram[c:c + 1, :], P))
                        invb = sc1.tile([P, Q], F32, tag="invb2", bufs=1)
                        nc.vector.reciprocal_approx_fast(invb, rmsb)
                        for kt in range(16):
                            nc.vector.tensor_mul(xn2[:, kt, :], x1[:, kt, :],
                                                 invb)
                            if kt % 8 == 7:
                                yield
                        g_pre.out[c] = (x1, xn2)
                    g_pre.out = {}

                    def g_gu(c):
                        """gate/up matmuls for chunk c (PE + weight DMA only;
                        no scalar work, so it interleaves with exp safely)."""
                        x1, xn2 = g_pre.out[c]
                        for iq in range(8):
                            wg_t = mlp.tile([P, 16, P], BF16, tag="wgt",
                                            bufs=2)
                            wu_t = mlp.tile([P, 16, P], BF16, tag="wut",
                                            bufs=2)
                            nc.sync.dma_start(
                                wg_t, wg_d[:, P * iq:P * (iq + 1)].rearrange(
                                    "(t p) m -> p t m", p=P))
                            nc.sync.dma_start(
                                wu_t, wu_d[:, P * iq:P * (iq + 1)].rearrange(
                                    "(t p) m -> p t m", p=P))
                            gps = ps[6][:, :]
                            ups = ps[7][:, :]
                            for kt in range(16):
                                st, sp = (kt == 0), (kt == 15)
                                nc.tensor.matmul(gps, wg_t[:, kt, :],
                                                 xn2[:, kt, :],
                                                 start=st, stop=sp)
                                nc.tensor.matmul(ups, wu_t[:, kt, :],
                                                 xn2[:, kt, :],
                                                 start=st, stop=sp)
                                if kt % 2 == 1:
                                    yield
                            # evict raw gate/up on DVE (table-neutral);
                            # silu happens batched in g_fin.
                            graw = mlp.tile([P, Q], BF16, tag=f"graw{iq}",
                                            bufs=1)
                            nc.vector.tensor_copy(graw, gps)
                            upr = mlp.tile([P, Q], BF16, tag=f"upr{iq}",
                                           bufs=1)
                            nc.vector.tensor_copy(upr, ups)
                            g_gu.out[(c, iq)] = (graw, upr)
                            yield
                    g_gu.out = {}

                    def g_fin(c):
                        """silu + hmlp, then down-proj + residual + out DMA."""
                        x1, _ = g_pre.out[c]
                        hmlp = mlp.tile([P, 8, Q], BF16, tag="hmlp", bufs=1)
                        for iq in range(8):
                            graw, upr = g_gu.out.pop((c, iq))
                            sg = sc1.tile([P, Q], F16, tag="sg", bufs=1)
                            nc.scalar.activation(sg, graw, AF.Silu)
                            nc.vector.tensor_mul(hmlp[:, iq, :], sg, upr)
                            if iq % 4 == 3:
                                yield
                        for hm in range(16):
                            dps = ps[2 + (hm % 2)][:, :]
                            for kt8 in range(8):
                                nc.tensor.matmul(
                                    dps,
                                    wd_all[:, kt8, P * hm:P * (hm + 1)],
                                    hmlp[:, kt8, :],
                                    start=(kt8 == 0), stop=(kt8 == 7))
                            dsb = sc1.tile([P, Q], F16, tag="dsb", bufs=2)
                            nc.vector.scalar_tensor_tensor(
                                dsb, x1[:, hm, :], 1.0 / N_CORES, dps,
                                op0=ALU.mult, op1=ALU.add)
                            nc.gpsimd.dma_start(
                                outT_d[P * hm:P * (hm + 1),
                                       Q * c:Q * (c + 1)], dsb)
                            yield

                    # ---- interleaved emission schedule ----
                    _drive(g_attn(0))
                    _drive(g_attn(1))
                    _drive(g_attn(2))
                    _drive(g_pre(0))
                    _drive(g_attn(3), g_gu(0))
                    _drive(g_pre(1))
                    _drive(g_fin(0), g_gu(1))
                    _drive(g_pre(2))
                    _drive(g_fin(1), g_gu(2))
                    _drive(g_pre(3))
                    _drive(g_fin(2), g_gu(3))
                    _drive(g_fin(3))

    nc.compile()
    return nc


_CACHE = {}


def _get_nc():
    if "nc" not in _CACHE:
        _CACHE["nc"] = build()
    return _CACHE["nc"]


def _prep_inputs(inputs):
    """Shard + preprocess full inputs into 8 per-core in_maps."""
    f = lambda k: np.asarray(inputs[k], dtype=np.float32)
    hidden = f("hidden_states")[0]                 # [S, HID]
    sin_t, cos_t = f("sin_table"), f("cos_table")  # [S, 32]
    ln1, ln2 = f("ln1_w"), f("ln2_w")
    wq = (f("wq") * ln1[:, None]).astype(np.float16)
    wk = (f("wk") * ln1[:, None]).astype(np.float16)
    wv = (f("wv") * ln1[:, None]).astype(np.float16)
    wo = f("wo").astype(np.float16)
    wg = (f("w_gate") * ln2[:, None]).astype(ml_dtypes.bfloat16)
    wu = (f("w_up") * ln2[:, None]).astype(ml_dtypes.bfloat16)
    wd = f("w_down").astype(ml_dtypes.bfloat16)

    hTb = np.ascontiguousarray(hidden.T).astype(np.float16)
    # rows per 64-block: [+sinT (x0 source); -sinT (x1 source)]
    sin4 = np.ascontiguousarray(
        np.tile(np.concatenate([sin_t.T, -sin_t.T], axis=0),
                (2, 1))).astype(np.float16)
    cos4 = np.ascontiguousarray(np.tile(cos_t.T, (4, 1))).astype(np.float16)
    ones = np.ones((P, 1), dtype=np.float32)
    cvec = np.concatenate(
        [np.full((P, 1), EPS, np.float32), np.full((P, 1), EXPB, np.float32)],
        axis=1)
    onesh = np.ones((P, 1), dtype=np.float16)
    rr = np.arange(P)[:, None]
    cols = np.arange(Q)[None, :]
    masks = np.concatenate(
        [(rr + 128 * t <= cols).astype(np.float32) for t in range(4)],
        axis=1).astype(np.float16)

    in_maps = []
    for c in range(N_CORES):
        qs = slice(QD * c, QD * (c + 1))
        ks = slice(HD * c, HD * (c + 1))
        isl = slice(INTER_LOC * c, INTER_LOC * (c + 1))
        in_maps.append({
            "hTb": hTb,
            "sin4": sin4,
            "cos4": cos4,
            "wq": np.ascontiguousarray(wq[:, qs]),
            "wkv": np.ascontiguousarray(
                np.concatenate([wk[:, ks], wv[:, ks]], axis=1)),
            "wo": np.ascontiguousarray(wo[qs, :]),
            "wg": np.ascontiguousarray(wg[:, isl]),
            "wu": np.ascontiguousarray(wu[:, isl]),
            "wd": np.ascontiguousarray(wd[isl, :]),
            "ones": ones,
            "cvec": cvec,
            "onesh": onesh,
            "masks": masks,
        })
    return in_maps


def kernel(**inputs):
    nc = _get_nc()
    in_maps = _prep_inputs(inputs)
    res = run_bass_kernel_spmd(nc, in_maps, core_ids=list(range(N_CORES)))
    acc = np.zeros((HID, S), dtype=np.float32)
    for c in range(N_CORES):
        acc += res.results[c]["outT"].astype(np.float32)
    return np.ascontiguousarray(acc.T)[None, :, :]


# revision 24
# speedup vs baseline: 1.0401x; 1.0100x over previous
"""Trainium2 Bass kernel for nn_DecoderLayer_66408784331382 (v4).

Single transformer decoder layer (RMSNorm + GQA attention w/ RoPE + RMSNorm +
SwiGLU MLP), tensor-parallel over 8 NeuronCores:

  - per core: 4 of 32 Q heads, 1 of 8 KV heads, 1024 of 8192 MLP inter cols,
    matching row-shards of wo / w_down.
  - activations transposed on device ([hid, tok]); host supplies x.T in fp16.
  - RMS1 folds *after* the QKV projection ((x*inv)@W == (x@W)*inv): QKV
    matmuls run on raw x; inv_rms folds into the RoPE cos/sin multipliers.
    The sum-of-squares stats for chunk c+1 are software-pipelined into chunk
    c's matmul stream so the inv_rms is ready the moment a chunk's PSUM
    accumulators complete — PSUM eviction never waits on the rms chain.
  - attention path is fp16, MLP bf16, PSUM accumulation fp32.  Softmax exp
    carries a -2.0 bias (cancels in normalization) for fp16 headroom.
  - V is laid out via XBAR DMA-transpose (no PE transposes / identity).
  - reciprocals use the approximate custom-DVE op on broadcast fp32 tiles.
  - the attention-output AllReduce is split into four 512-token fp16 chunks;
    a dummy 1-element AllReduce at kernel start absorbs the collective
    warm-up latency.  Emission is interleaved via generators: gate/up
    matmuls of MLP chunk c fill the PE while attention of later quarters
    waits on the exp chain; down-proj of chunk c overlaps gate/up of c+1.
  - o-proj accumulates across a 4-bank PSUM rotation with DVE evictions.
  - DMA queues: sync = streaming loads, scalar = weight preamble, vector =
    phase-1 rms broadcasts, gpsimd = gated traffic (collectives, softmax
    denominators, outputs).
  - down-proj partials (+ x1/8 residual) are written fp16, summed on host.

kernel(**inputs) takes the FULL fp32 inputs of reference.setup_inputs() and
returns the FULL [1, 2048, 2048] fp32 output.
"""

import sys

if "/opt/trn_rl_repo" not in sys.path:
    sys.path.insert(0, "/opt/trn_rl_repo")

import numpy as np
import ml_dtypes

import concourse.bass as bass
import concourse.mybir as mybir
import concourse.tile as tile
from concourse import bacc
from concourse.bass_utils import run_bass_kernel_spmd

# ---- problem constants (hardcoded per contract) ----
N_CORES = 8
S = 2048
HID = 2048
HD = 64
NH = 32
INTER = 8192
EPS = 1e-6

QD = (NH // N_CORES) * HD        # 256 local q cols (2 tiles of 128)
INTER_LOC = INTER // N_CORES     # 1024
SCALE = 1.0 / np.sqrt(HD)
EXPB = -2.0                      # softmax exp bias (cancels in normalization)

F32 = mybir.dt.float32
F32R = mybir.dt.float32r
BF16 = mybir.dt.bfloat16
F16 = mybir.dt.float16

P = 128
Q = 512      # token quarter (phase-1 chunk, attention block, MLP chunk)
NQ = S // Q  # 4
ARDT = F16   # collective dtype
AF = mybir.ActivationFunctionType
ALU = mybir.AluOpType


def _bcast(ap, parts):
    """View a [1, N] AP as [parts, N] via partition-stride-0 (DMA broadcast)."""
    return bass.AP(tensor=ap.tensor, offset=ap.offset,
                   ap=[[0, parts]] + [list(p) for p in ap.ap[1:]])


def _drive(*gens):
    """Round-robin the emission generators until all are exhausted."""
    active = [g for g in gens if g is not None]
    while active:
        for g in list(active):
            try:
                next(g)
            except StopIteration:
                active.remove(g)


def build():
    nc = bacc.Bacc("TRN2", target_bir_lowering=False, debug=False,
                   num_devices=N_CORES)

    hTb_d = nc.dram_tensor("hTb", [HID, S], F16, kind="ExternalInput")
    sin4_d = nc.dram_tensor("sin4", [P, S], F16, kind="ExternalInput")
    cos4_d = nc.dram_tensor("cos4", [P, S], F16, kind="ExternalInput")
    wq_d = nc.dram_tensor("wq", [HID, QD], F16, kind="ExternalInput")
    wkv_d = nc.dram_tensor("wkv", [HID, 2 * HD], F16, kind="ExternalInput")
    wo_d = nc.dram_tensor("wo", [QD, HID], F16, kind="ExternalInput")
    wg_d = nc.dram_tensor("wg", [HID, INTER_LOC], BF16, kind="ExternalInput")
    wu_d = nc.dram_tensor("wu", [HID, INTER_LOC], BF16, kind="ExternalInput")
    wd_d = nc.dram_tensor("wd", [INTER_LOC, HID], BF16, kind="ExternalInput")
    ones_d = nc.dram_tensor("ones", [P, 1], F32R, kind="ExternalInput")
    cvec_d = nc.dram_tensor("cvec", [P, 2], F32, kind="ExternalInput")
    masks_d = nc.dram_tensor("masks", [P, 4 * Q], F16, kind="ExternalInput")
    onesh_d = nc.dram_tensor("onesh", [P, 1], F16, kind="ExternalInput")
    outT_d = nc.dram_tensor("outT", [HID, S], F16, kind="ExternalOutput")

    with tile.TileContext(nc) as tc, nc.allow_low_precision(
            reason="f16/bf16 activations within a 2e-2 rel-err budget"):
        with (
            tc.tile_pool(name="const", bufs=1) as const,
            tc.tile_pool(name="dramp", bufs=1, space="DRAM") as dram,
        ):
            ones1 = const.tile([P, 1], F32R)
            eps1 = const.tile([P, 1], F32)
            expb1 = const.tile([P, 1], F32)
            nc.scalar.dma_start(ones1, ones_d[:, :])
            nc.scalar.dma_start(eps1, cvec_d[:, 0:1])
            nc.scalar.dma_start(expb1, cvec_d[:, 1:2])

            ar_in = [dram.tile([HID, Q], ARDT, name=f"ar_in{i}",
                               tag=f"ar_in{i}") for i in range(NQ)]
            ar_out = [dram.tile([HID, Q], ARDT, addr_space="Shared",
                                name=f"ar_out{i}", tag=f"ar_out{i}")
                      for i in range(NQ)]
            warm_in = dram.tile([P, Q], ARDT, tag="warm_in")
            warm_out = dram.tile([P, Q], ARDT, addr_space="Shared",
                                 tag="warm_out")
            bc1_dram = dram.tile([NQ, Q], F32)           # phase-1 rms rows
            bc2_dram = dram.tile([NQ, 2, 2, Q], F32)     # softmax denoms
            bc4_dram = dram.tile([NQ, Q], F32)           # rms2 rows

            # warm up the collective pipeline while phase 1 computes
            nc.gpsimd.collective_compute(
                "AllReduce", ALU.add,
                replica_groups=[list(range(N_CORES))],
                ins=[warm_in[:, :].opt()],
                outs=[warm_out[:, :].opt()])

            # ======== persistent tensors ===================================
            with tc.tile_pool(name="keep", bufs=1) as keep:
                masks = keep.tile([P, 4, Q], F16)
                nc.scalar.dma_start(
                    masks, masks_d[:, :].rearrange("p (t n) -> p t n", t=4))
                qT = [keep.tile([P, S], F16, tag=f"qT{m}", name=f"qT{m}")
                      for m in range(2)]
                kTdup = keep.tile([P, S], F16, tag="kTdup")
                v_ones = keep.tile([P, 16, HD + 1], F16, tag="v_ones")
                nc.scalar.dma_start(
                    v_ones[:, :, HD:HD + 1],
                    _bcast(bass.AP(tensor=onesh_d.tensor
                                   if hasattr(onesh_d, "tensor") else onesh_d,
                                   offset=0, ap=[[0, 1], [0, 16], [0, 1]]),
                           P))
                wo_all = keep.tile([P, 2, HID], F16, tag="wo_all")
                wd_all = keep.tile([P, 8, HID], BF16, tag="wd_all")

                # ---- Phase 1: QKV + pipelined RMS1 stats + RoPE -----------
                with (
                    tc.tile_pool(name="p1w", bufs=1) as p1w,
                    tc.tile_pool(name="p1x", bufs=2) as p1x,
                    tc.tile_pool(name="p1s", bufs=1) as p1s,
                    tc.tile_pool(name="p1ps", bufs=2, space="PSUM") as p1ps,
                ):
                    wq_all = p1w.tile([P, 16, QD], F16)
                    wkv_all = p1w.tile([P, 16, 2 * HD], F16)
                    nc.scalar.dma_start(
                        wq_all, wq_d[:, :].rearrange("(t p) m -> p t m", p=P))
                    nc.scalar.dma_start(
                        wkv_all, wkv_d[:, :].rearrange("(t p) m -> p t m", p=P))
                    sin4 = p1w.tile([P, S], F16)
                    cos4 = p1w.tile([P, S], F16)
                    nc.scalar.dma_start(sin4, sin4_d[:, :])
                    nc.scalar.dma_start(cos4, cos4_d[:, :])
                    xqs, effs = {}, {}

                    def load_xq(c):
                        t = p1x.tile([P, 16, Q], F16, tag="xq")
                        cc = slice(Q * c, Q * (c + 1))
                        for t4 in range(4):
                            nc.sync.dma_start(
                                t[:, 4 * t4:4 * (t4 + 1), :],
                                hTb_d[512 * t4:512 * (t4 + 1), cc].rearrange(
                                    "(t p) m -> p t m", p=P))
                        xqs[c] = t

                    def stats_sq(c, kt, ssq):
                        """Square + ones-matmul accumulate for (c, kt)."""
                        sq = p1s.tile([P, Q], F32R, tag="sq", bufs=3)
                        nc.scalar.activation(sq, xqs[c][:, kt, :], AF.Square)
                        nc.tensor.matmul(ssq, ones1, sq,
                                         start=(kt == 0), stop=(kt == 15))

                    def rms_chain(c, ssq):
                        """Sqrt -> broadcast round-trip (vector queue) ->
                        approx reciprocal -> effective cos/sin."""
                        rms = p1s.tile([1, Q], F32, tag="rms", bufs=2)
                        nc.scalar.activation(rms, ssq, AF.Sqrt,
                                             bias=eps1[0:1, :], scale=1.0 / HID)
                        nc.sync.dma_start(bc1_dram[c:c + 1, :], rms)
                        rmsb = p1s.tile([P, Q], F32, tag="rmsb", bufs=2)
                        nc.sync.dma_start(rmsb,
                                          _bcast(bc1_dram[c:c + 1, :], P))
                        invb = p1s.tile([P, Q], F32, tag="invb", bufs=2)
                        nc.vector.reciprocal_approx_fast(invb, rmsb)
                        cc = slice(Q * c, Q * (c + 1))
                        cos_e = p1s.tile([P, Q], F16, tag="cos_e", bufs=2)
                        sin_e = p1s.tile([P, Q], F16, tag="sin_e", bufs=2)
                        nc.vector.tensor_mul(cos_e, cos4[:, cc], invb)
                        nc.vector.tensor_mul(sin_e, sin4[:, cc], invb)
                        effs[c] = (cos_e, sin_e, invb)

                    # pipeline fill: chunk-0 stats alone
                    load_xq(0)
                    load_xq(1)
                    ssq_c = p1ps.tile([1, Q], F32, tag="ssq")
                    for kt in range(16):
                        stats_sq(0, kt, ssq_c)
                    rms_chain(0, ssq_c)

                    for c in range(NQ):
                        cc = slice(Q * c, Q * (c + 1))
                        qm = p1ps.tile([P, 2 * Q], F32, tag="qm")
                        kv_ps = p1ps.tile([P, Q], F32, tag="kvps")
                        if c + 1 < NQ:
                            ssq_n = p1ps.tile([1, Q], F32, tag="ssq")
                        for kt in range(16):
                            st, sp = (kt == 0), (kt == 15)
                            xt = xqs[c][:, kt, :]
                            for m in range(2):
                                nc.tensor.matmul(
                                    qm[:, Q * m:Q * (m + 1)],
                                    wq_all[:, kt, P * m:P * (m + 1)],
                                    xt, start=st, stop=sp)
                            nc.tensor.matmul(kv_ps, wkv_all[:, kt, :],
                                             xt, start=st, stop=sp)
                            if c + 1 < NQ:
                                stats_sq(c + 1, kt, ssq_n)
                        if c + 2 < NQ:
                            load_xq(c + 2)
                        if c + 1 < NQ:
                            rms_chain(c + 1, ssq_n)
                        del xqs[c]

                        # RoPE eviction (inv_rms is ready: stats pipelined)
                        cos_e, sin_e, invb = effs.pop(c)
                        for m in range(2):
                            qp = qm[:, Q * m:Q * (m + 1)]
                            s1 = p1s.tile([P, Q], F16, tag="s1", bufs=2)
                            s2 = p1s.tile([P, Q], F16, tag="s2", bufs=2)
                            nc.vector.tensor_mul(s1, qp, cos_e)
                            for b in range(2):
                                x0 = slice(64 * b, 64 * b + 32)
                                x1s = slice(64 * b + 32, 64 * b + 64)
                                nc.vector.tensor_mul(
                                    s2[x0, :], qp[x1s, :], sin_e[x1s, :])
                                nc.vector.tensor_mul(
                                    s2[x1s, :], qp[x0, :], sin_e[x0, :])
                            nc.vector.tensor_add(qT[m][:, cc], s1, s2)
                        s1 = p1s.tile([64, Q], F16, tag="s1k", bufs=2)
                        s2 = p1s.tile([64, Q], F16, tag="s2k", bufs=2)
                        nc.vector.tensor_mul(s1, kv_ps[0:64, :],
                                             cos_e[0:64, :])
                        nc.vector.tensor_mul(
                            s2[0:32, :], kv_ps[32:64, :], sin_e[32:64, :])
                        nc.vector.tensor_mul(
                            s2[32:64, :], kv_ps[0:32, :], sin_e[0:32, :])
                        nc.vector.tensor_add(kTdup[0:64, cc], s1, s2)
                        nc.vector.tensor_copy(kTdup[64:128, cc],
                                              kTdup[0:64, cc])
                        # v: scale by inv, then XBAR DMA-transpose
                        vt = p1s.tile([64, Q], F16, tag="vt", bufs=2)
                        nc.vector.tensor_mul(vt, kv_ps[64:128, :],
                                             invb[0:64, :])
                        for j in range(Q // P):
                            vst = p1s.tile([P, HD], F16, tag="vst", bufs=2)
                            nc.sync.dma_start_transpose(
                                vst, vt[:, P * j:P * (j + 1)])
                            nc.vector.tensor_copy(
                                v_ones[:, (Q // P) * c + j, 0:HD], vst)

                nc.scalar.dma_start(
                    wo_all, wo_d[:, :].rearrange("(t p) m -> p t m", p=P))
                nc.scalar.dma_start(
                    wd_all, wd_d[:, :].rearrange("(t p) m -> p t m", p=P))

                # ---- Phases 2-4, interleaved via emission generators ------
                with (
                    tc.tile_pool(name="ps", bufs=1, space="PSUM") as psp,
                    tc.tile_pool(name="att", bufs=2) as att,
                    tc.tile_pool(name="mlp", bufs=2) as mlp,
                    tc.tile_pool(name="sc1", bufs=1) as sc1,
                ):
                    ps = [psp.tile([P, Q], F32, tag=f"ps{i}", name=f"ps{i}")
                          for i in range(8)]

                    def g_attn(qc4):
                        """Scores/softmax/PV per m, then o-proj, then AR."""
                        qs = slice(Q * qc4, Q * (qc4 + 1))
                        atn = []
                        for m in range(2):
                            pv = [ps[2 + 2 * m][0:HD + 1, :],
                                  ps[3 + 2 * m][0:HD + 1, :]]
                            nkt = 4 * qc4 + 4
                            for kt in range(nkt):
                                st, sp = (kt == 0), (kt == nkt - 1)
                                for b in range(2):
                                    rows = slice(64 * b, 64 * (b + 1))
                                    sc = ps[b][:, :]
                                    nc.tensor.matmul(
                                        sc,
                                        kTdup[rows, P * kt:P * (kt + 1)],
                                        qT[m][rows, qs],
                                        start=True, stop=True)
                                    pr = att.tile([P, Q], F16, tag=f"pr{b}",
                                                  bufs=2)
                                    nc.scalar.activation(
                                        pr, sc, AF.Exp, bias=expb1,
                                        scale=float(SCALE))
                                    if kt >= 4 * qc4:
                                        nc.vector.tensor_mul(
                                            pr, pr,
                                            masks[:, kt - 4 * qc4, :])
                                    nc.tensor.matmul(
                                        pv[b], v_ones[:, kt, :], pr,
                                        start=st, stop=sp)
                                yield
                            at = att.tile([P, Q], F16, tag=f"atn{m}")
                            atn.append(at)
                            for b in range(2):
                                den = att.tile([1, Q], F32, tag=f"den{b}")
                                nc.scalar.copy(den, pv[b][HD:HD + 1, :])
                                slot = bc2_dram[qc4:qc4 + 1, m, b, :]
                                nc.sync.dma_start(slot, den)
                                recb = att.tile([64, Q], F32, tag=f"recb{b}",
                                                bufs=1)
                                nc.sync.dma_start(recb, _bcast(slot, 64))
                                rec = att.tile([64, Q], F32, tag=f"rec{b}",
                                               bufs=1)
                                nc.vector.reciprocal_approx_fast(rec, recb)
                                nc.vector.tensor_mul(
                                    at[64 * b:64 * (b + 1), :],
                                    pv[b][0:HD, :], rec)
                            yield
                        for hm in range(16):
                            ops = ps[hm % 4][:, :]
                            for kt2 in range(2):
                                nc.tensor.matmul(
                                    ops,
                                    wo_all[:, kt2, P * hm:P * (hm + 1)],
                                    atn[kt2],
                                    start=(kt2 == 0), stop=(kt2 == 1))
                            osb = att.tile([P, Q], ARDT, tag="osb", bufs=2)
                            nc.vector.tensor_copy(osb, ops)
                            nc.sync.dma_start(
                                ar_in[qc4][P * hm:P * (hm + 1), :], osb)
                            if hm % 2 == 1:
                                yield
                        nc.gpsimd.collective_compute(
                            "AllReduce", ALU.add,
                            replica_groups=[list(range(N_CORES))],
                            ins=[ar_in[qc4][:, :].opt()],
                            outs=[ar_out[qc4][:, :].opt()])

                    def g_pre(c):
                        """x1 = x + attn (f16), rms2 stats -> invb -> xn2."""
                        cs = slice(Q * c, Q * (c + 1))
                        x1 = mlp.tile([P, 16, Q], F16, tag="x1")
                        xn2 = mlp.tile([P, 16, Q], BF16, tag="xn2")
                        ssq2 = ps[4][0:1, :]
                        for kt in range(16):
                            rs = slice(P * kt, P * (kt + 1))
                            th = sc1.tile([P, Q], F16, tag="th", bufs=2)
                            ta = sc1.tile([P, Q], ARDT, tag="ta", bufs=2)
                            nc.sync.dma_start(th, hTb_d[rs, cs])
                            nc.gpsimd.dma_start(ta, ar_out[c][rs, :])
                            nc.vector.tensor_add(x1[:, kt, :], th, ta)
                            sq = sc1.tile([P, Q], F32R, tag="sq2", bufs=2)
                            nc.scalar.activation(sq, x1[:, kt, :], AF.Square)
                            nc.tensor.matmul(ssq2, ones1, sq,
                                             start=(kt == 0), stop=(kt == 15))
                            if kt % 4 == 3:
                                yield
                        rms = sc1.tile([1, Q], F32, tag="rms2", bufs=2)
                        nc.scalar.activation(rms, ssq2, AF.Sqrt,
                                             bias=eps1[0:1, :], scale=1.0 / HID)
                        nc.gpsimd.dma_start(bc4_dram[c:c + 1, :], rms)
                        rmsb = sc1.tile([P, Q], F32, tag="rmsb2", bufs=1)
                        nc.gpsimd.dma_start(rmsb,
                                            _bcast(bc4_dram[c:c + 1, :], P))
                        invb = sc1.tile([P, Q], F32, tag="invb2", bufs=1)
                        nc.vector.reciprocal_approx_fast(invb, rmsb)
                        for kt in range(16):
                            nc.vector.tensor_mul(xn2[:, kt, :], x1[:, kt, :],
                                                 invb)
                            if kt % 8 == 7:
                                yield
                        g_pre.out[c] = (x1, xn2)
                    g_pre.out = {}

                    def g_gu(c):
                        """gate/up matmuls for chunk c (PE + weight DMA only;
                        no scalar work, so it interleaves with exp safely)."""
                        x1, xn2 = g_pre.out[c]
                        for iq in range(8):
                            wg_t = mlp.tile([P, 16, P], BF16, tag="wgt",
                                            bufs=2)
                            wu_t = mlp.tile([P, 16, P], BF16, tag="wut",
                                            bufs=2)
                            nc.sync.dma_start(
                                wg_t, wg_d[:, P * iq:P * (iq + 1)].rearrange(
                                    "(t p) m -> p t m", p=P))
                            nc.sync.dma_start(
                                wu_t, wu_d[:, P * iq:P * (iq + 1)].rearrange(
                                    "(t p) m -> p t m", p=P))
                            gps = ps[6][:, :]
                            ups = ps[7][:, :]
                            for kt in range(16):
                                st, sp = (kt == 0), (kt == 15)
                                nc.tensor.matmul(gps, wg_t[:, kt, :],
                                                 xn2[:, kt, :],
                                                 start=st, stop=sp)
                                nc.tensor.matmul(ups, wu_t[:, kt, :],
                                                 xn2[:, kt, :],
                                                 start=st, stop=sp)
                                if kt % 2 == 1:
                                    yield
                            # evict raw gate/up on DVE (table-neutral);
                            # silu happens batched in g_fin.
                            graw = mlp.tile([P, Q], BF16, tag=f"graw{iq}",
                                            bufs=1)
                            nc.vector.tensor_copy(graw, gps)
                            upr = mlp.tile([P, Q], BF16, tag=f"upr{iq}",
                                           bufs=1)
                            nc.vector.tensor_copy(upr, ups)
                            g_gu.out[(c, iq)] = (graw, upr)
                            yield
                    g_gu.out = {}

                    def g_fin(c):
                        """silu + hmlp, then down-proj + residual + out DMA."""
                        x1, _ = g_pre.out[c]
                        hmlp = mlp.tile([P, 8, Q], BF16, tag="hmlp", bufs=1)
                        for iq in range(8):
                            graw, upr = g_gu.out.pop((c, iq))
                            sg = sc1.tile([P, Q], F16, tag="sg", bufs=1)
                            nc.scalar.activation(sg, graw, AF.Silu)
                            nc.vector.tensor_mul(hmlp[:, iq, :], sg, upr)
                            if iq % 4 == 3:
                                yield
                        for hm in range(16):
                            dps = ps[2 + (hm % 2)][:, :]
                            for kt8 in range(8):
                                nc.tensor.matmul(
                                    dps,
                                    wd_all[:, kt8, P * hm:P * (hm + 1)],
                                    hmlp[:, kt8, :],
                                    start=(kt8 == 0), stop=(kt8 == 7))
                            dsb = sc1.tile([P, Q], F16, tag="dsb", bufs=2)
                            nc.vector.scalar_tensor_tensor(
                                dsb, x1[:, hm, :], 1.0 / N_CORES, dps,
                                op0=ALU.mult, op1=ALU.add)
                            nc.gpsimd.dma_start(
                                outT_d[P * hm:P * (hm + 1),
                                       Q * c:Q * (c + 1)], dsb)
                            yield

                    # ---- interleaved emission schedule ----
                    _drive(g_attn(0))
                    _drive(g_attn(1))
                    _drive(g_attn(2))
                    _drive(g_pre(0))
                    _drive(g_attn(3), g_gu(0))
                    _drive(g_pre(1))
                    _drive(g_fin(0), g_gu(1))
                    _drive(g_pre(2))
                    _drive(g_fin(1), g_gu(2))
                    _drive(g_pre(3))
                    _drive(g_fin(2), g_gu(3))
                    _drive(g_fin(3))

    nc.compile()
    return nc


_CACHE = {}


def _get_nc():
    if "nc" not in _CACHE:
        _CACHE["nc"] = build()
    return _CACHE["nc"]


def _prep_inputs(inputs):
    """Shard + preprocess full inputs into 8 per-core in_maps."""
    f = lambda k: np.asarray(inputs[k], dtype=np.float32)
    hidden = f("hidden_states")[0]                 # [S, HID]
    sin_t, cos_t = f("sin_table"), f("cos_table")  # [S, 32]
    ln1, ln2 = f("ln1_w"), f("ln2_w")
    wq = (f("wq") * ln1[:, None]).astype(np.float16)
    wk = (f("wk") * ln1[:, None]).astype(np.float16)
    wv = (f("wv") * ln1[:, None]).astype(np.float16)
    wo = f("wo").astype(np.float16)
    wg = (f("w_gate") * ln2[:, None]).astype(ml_dtypes.bfloat16)
    wu = (f("w_up") * ln2[:, None]).astype(ml_dtypes.bfloat16)
    wd = f("w_down").astype(ml_dtypes.bfloat16)

    hTb = np.ascontiguousarray(hidden.T).astype(np.float16)
    # rows per 64-block: [+sinT (x0 source); -sinT (x1 source)]
    sin4 = np.ascontiguousarray(
        np.tile(np.concatenate([sin_t.T, -sin_t.T], axis=0),
                (2, 1))).astype(np.float16)
    cos4 = np.ascontiguousarray(np.tile(cos_t.T, (4, 1))).astype(np.float16)
    ones = np.ones((P, 1), dtype=np.float32)
    cvec = np.concatenate(
        [np.full((P, 1), EPS, np.float32), np.full((P, 1), EXPB, np.float32)],
        axis=1)
    onesh = np.ones((P, 1), dtype=np.float16)
    rr = np.arange(P)[:, None]
    cols = np.arange(Q)[None, :]
    masks = np.concatenate(
        [(rr + 128 * t <= cols).astype(np.float32) for t in range(4)],
        axis=1).astype(np.float16)

    in_maps = []
    for c in range(N_CORES):
        qs = slice(QD * c, QD * (c + 1))
        ks = slice(HD * c, HD * (c + 1))
        isl = slice(INTER_LOC * c, INTER_LOC * (c + 1))
        in_maps.append({
            "hTb": hTb,
            "sin4": sin4,
            "cos4": cos4,
            "wq": np.ascontiguousarray(wq[:, qs]),
            "wkv": np.ascontiguousarray(
                np.concatenate([wk[:, ks], wv[:, ks]], axis=1)),
            "wo": np.ascontiguousarray(wo[qs, :]),
            "wg": np.ascontiguousarray(wg[:, isl]),
            "wu": np.ascontiguousarray(wu[:, isl]),
            "wd": np.ascontiguousarray(wd[isl, :]),
            "ones": ones,
            "cvec": cvec,
            "onesh": onesh,
            "masks": masks,
        })
    return in_maps


def kernel(**inputs):
    nc = _get_nc()
    in_maps = _prep_inputs(inputs)
    res = run_bass_kernel_spmd(nc, in_maps, core_ids=list(range(N_CORES)))
    acc = np.zeros((HID, S), dtype=np.float32)
    for c in range(N_CORES):
        acc += res.results[c]["outT"].astype(np.float32)
    return np.ascontiguousarray(acc.T)[None, :, :]
